# revision 9
# baseline (speedup 1.0000x reference)
"""Trainium2 Bass kernel for 3-relation HeteroGAT (edge softmax + scatter).

Strategy (dst-sharded, payload-shipping):
  - Host shards the 200K dst tasks across 8 cores (25K each), sorts each
    relation's edges by dst, pads each (128-dst tile, relation) group to a
    static per-tile block count, and packs per-edge payloads
    [x_src | x_dst | eattr] (replicated node features) in [p, b] layout.
    Host does layout/permutation only - zero arithmetic.
  - Device computes per-edge logits with one fused dot against a
    device-computed weight vector [Ws@as | Wd@ad | We@ae], leaky_relu+exp on
    ACT, builds one-hot (dstslot vs iota) on DVE, and accumulates
    A^T[feat, dstslot] += [ex*x_src | ex] per block with PE matmuls into
    PSUM (the softmax denominator rides along as the last feature row).
    Per tile: out = (A[:, :Ds] @ Ws) / denom + x@Wr + b, then LayerNorm and
    leaky_relu. No collectives needed: every dst's edges live on one core.
"""

import sys
import numpy as np

for _p in ("/opt/trn_rl_repo", "/root/.axon_site/_ro/trn_rl_repo"):
    if _p not in sys.path:
        sys.path.append(_p)

import concourse.bass as bass
import concourse.mybir as mybir
import concourse.tile as tile
from concourse import bacc, bass_utils

F32 = mybir.dt.float32
F16 = mybir.dt.float16

N_TASKS, N_DATA, N_DEV = 200000, 100000, 4
C = 64
NCORES = 8
PT = N_TASKS // NCORES          # dst tasks per core
TILES = (PT + 127) // 128       # 196 (last tile partial: 25000 = 195*128 + 40)
TPAD = TILES * 128              # 25088
DOUT = 12 + 3 * C               # 204
RT = 4                          # tiles per round
assert TILES % RT == 0

# relation static config: (name, Ds_src_feats, de_edge_feats)
RELS = [("td", 5, 3), ("tt", 12, 1), ("dv", 12, 2)]


# --------------------------------------------------------------------------
# host-side packing (layout only, no arithmetic)
# --------------------------------------------------------------------------

def _pack_relation(src, dst, eattr, x_src_tab, x_tasks, Ds, de):
    """Returns (xcat [8,128,TILES*NB*Dcat] f32, dslot [8,128,TILES*NB] f16, NB)."""
    E = src.shape[0]
    Dcat = Ds + 12 + de
    core = dst // PT
    local = dst - core * PT
    tl = local >> 7
    slot = local & 127
    key = core * TILES + tl
    order = np.argsort(key, kind="stable")
    cnt = np.bincount(key, minlength=NCORES * TILES)
    NB = max(1, int(-(-cnt.max() // 128)))
    cap = NB * 128
    gstart = np.zeros(NCORES * TILES, dtype=np.int64)
    gstart[1:] = np.cumsum(cnt)[:-1]
    okey = key[order]
    rank = np.arange(E, dtype=np.int64) - gstart[okey]
    pos = okey * cap + rank

    xcat = np.zeros((NCORES * TILES * cap, Dcat), dtype=np.float32)
    osrc, odst = src[order], dst[order]
    xcat[pos, 0:Ds] = x_src_tab[osrc]
    xcat[pos, Ds:Ds + 12] = x_tasks[odst]
    xcat[pos, Ds + 12:] = eattr[order]
    dslot = np.full(NCORES * TILES * cap, -1.0, dtype=np.float16)
    dslot[pos] = slot[order].astype(np.float16)

    xcat = (xcat.reshape(NCORES, TILES * NB, 128, Dcat)
            .transpose(0, 2, 1, 3).reshape(NCORES, 128, TILES * NB * Dcat))
    dslot = (dslot.reshape(NCORES, TILES * NB, 128)
             .transpose(0, 2, 1).reshape(NCORES, 128, TILES * NB))
    return np.ascontiguousarray(xcat), np.ascontiguousarray(dslot), NB


def _prep_inputs(inp):
    """Build per-core in_maps. Pure layout/concat of inputs + constants."""
    x_tasks = np.asarray(inp["x_tasks"], dtype=np.float32)
    x_data = np.asarray(inp["x_data"], dtype=np.float32)
    x_dev = np.asarray(inp["x_devices"], dtype=np.float32)
    src_tabs = {"td": x_data, "tt": x_tasks, "dv": x_dev}

    packed = {}
    NBs = {}
    for name, Ds, de in RELS:
        xc, dsl, nb = _pack_relation(
            np.asarray(inp[f"src_{name}"]), np.asarray(inp[f"dst_{name}"]),
            np.asarray(inp[f"eattr_{name}"], dtype=np.float32),
            src_tabs[name], x_tasks, Ds, de)
        packed[name] = (xc, dsl)
        NBs[name] = nb

    # x_tasks slice per core, padded to TPAD rows
    x_pad = np.zeros((NCORES, TPAD, 12), dtype=np.float32)
    for k in range(NCORES):
        x_pad[k, :PT] = x_tasks[k * PT:(k + 1) * PT]

    # rhs_resid [13, 204]: rows0:12 = [I12 | Wr_td | Wr_tt | Wr_dv], row12 = biases
    rhs_resid = np.zeros((13, DOUT), dtype=np.float32)
    rhs_resid[0:12, 0:12] = np.eye(12, dtype=np.float32)
    for i, (name, _, _) in enumerate(RELS):
        c0 = 12 + 64 * i
        rhs_resid[0:12, c0:c0 + 64] = np.asarray(inp[f"Wr_{name}"], np.float32)
        rhs_resid[12, c0:c0 + 64] = np.asarray(inp[f"b_{name}"], np.float32)

    const = {
        "rhs_resid": rhs_resid,
        "g_rep": np.tile(np.asarray(inp["ln_g"], np.float32)[None, :], (128, 1)),
        "b_rep": np.tile(np.asarray(inp["ln_b"], np.float32)[None, :], (128, 1)),
        "iota_rep": np.tile(np.arange(128, dtype=np.float16)[None, :], (128, 1)),
        "ident": np.eye(128, dtype=np.float32),
    }
    for name, Ds, de in RELS:
        # rhs_ext [Ds+1, 65]: rows0:Ds = Ws, col64 = denominator indicator
        re_ = np.zeros((Ds + 1, 65), dtype=np.float32)
        re_[0:Ds, 0:64] = np.asarray(inp[f"Ws_{name}"], np.float32)
        re_[Ds, 64] = 1.0
        const[f"rhs_ext_{name}"] = re_
        const[f"WsT_{name}"] = np.ascontiguousarray(
            np.asarray(inp[f"Ws_{name}"], np.float32).T)          # [64, Ds]
        const[f"WdT_{name}"] = np.ascontiguousarray(
            np.asarray(inp[f"Wd_{name}"], np.float32).T)          # [64, 12]
        const[f"WeT_{name}"] = np.ascontiguousarray(
            np.asarray(inp[f"We_{name}"], np.float32).T)          # [64, de]
        const[f"avec_{name}"] = np.stack(
            [np.asarray(inp[f"as_{name}"], np.float32),
             np.asarray(inp[f"ad_{name}"], np.float32),
             np.asarray(inp[f"ae_{name}"], np.float32)], axis=1)  # [64, 3]

    in_maps = []
    for k in range(NCORES):
        m = {"x_pad": x_pad[k]}
        for name, _, _ in RELS:
            m[f"xcat_{name}"] = packed[name][0][k]
            m[f"dslot_{name}"] = packed[name][1][k]
        m.update(const)
        in_maps.append(m)
    return in_maps, NBs


# --------------------------------------------------------------------------
# device program
# --------------------------------------------------------------------------

def build_program(NBs, n_tiles=None, rt=None):
    n_tiles = TILES if n_tiles is None else n_tiles
    rt = RT if rt is None else rt
    nc = bacc.Bacc("TRN2", target_bir_lowering=False, debug=False,
                   num_devices=NCORES)
    AL = mybir.AluOpType
    AF = mybir.ActivationFunctionType

    p_x = nc.declare_dram_parameter("x_pad", [n_tiles * 128, 12], F32, isOutput=False)
    p_xcat, p_dslot = {}, {}
    for name, Ds, de in RELS:
        Dcat = Ds + 12 + de
        nb = NBs[name]
        p_xcat[name] = nc.declare_dram_parameter(
            f"xcat_{name}", [128, n_tiles * nb * Dcat], F32, isOutput=False)
        p_dslot[name] = nc.declare_dram_parameter(
            f"dslot_{name}", [128, n_tiles * nb], F16, isOutput=False)
    p_rr = nc.declare_dram_parameter("rhs_resid", [13, DOUT], F32, isOutput=False)
    p_g = nc.declare_dram_parameter("g_rep", [128, DOUT], F32, isOutput=False)
    p_b = nc.declare_dram_parameter("b_rep", [128, DOUT], F32, isOutput=False)
    p_iota = nc.declare_dram_parameter("iota_rep", [128, 128], F16, isOutput=False)
    p_id = nc.declare_dram_parameter("ident", [128, 128], F32, isOutput=False)
    p_cst = {}
    for name, Ds, de in RELS:
        p_cst[name] = {
            "rhs_ext": nc.declare_dram_parameter(f"rhs_ext_{name}", [Ds + 1, 65], F32, isOutput=False),
            "WsT": nc.declare_dram_parameter(f"WsT_{name}", [64, Ds], F32, isOutput=False),
            "WdT": nc.declare_dram_parameter(f"WdT_{name}", [64, 12], F32, isOutput=False),
            "WeT": nc.declare_dram_parameter(f"WeT_{name}", [64, de], F32, isOutput=False),
            "avec": nc.declare_dram_parameter(f"avec_{name}", [64, 3], F32, isOutput=False),
        }
    p_out = nc.declare_dram_parameter("out", [n_tiles * 128, DOUT], F32, isOutput=True)

    with tile.TileContext(nc) as tc:
        with (
            tc.tile_pool(name="const", bufs=1) as cpool,
            tc.tile_pool(name="round", bufs=2) as rpool,
            tc.tile_pool(name="tl", bufs=2) as tpool,
        ):
            # ---------------- setup ----------------
            spsp_cm = tc.tile_pool(name="pss", bufs=1, space="PSUM")
            spsp = spsp_cm.__enter__()
            iota_sb = cpool.tile([128, 128], F16, tag="iota")
            nc.sync.dma_start(out=iota_sb[:], in_=p_iota[:])
            ident_sb = cpool.tile([128, 128], F32, tag="ident")
            nc.sync.dma_start(out=ident_sb[:], in_=p_id[:])
            g_sb = cpool.tile([128, DOUT], F32, tag="g")
            nc.sync.dma_start(out=g_sb[:], in_=p_g[:])
            b_sb = cpool.tile([128, DOUT], F32, tag="b")
            nc.sync.dma_start(out=b_sb[:], in_=p_b[:])
            rr_f = cpool.tile([13, DOUT], F32, tag="rrf")
            nc.sync.dma_start(out=rr_f[:], in_=p_rr[:])
            rr_sb = cpool.tile([13, DOUT], F16, tag="rr")
            nc.vector.tensor_copy(out=rr_sb[:], in_=rr_f[:])

            wvec_rep, rhs_ext = {}, {}
            for name, Ds, de in RELS:
                Dcat = Ds + 12 + de
                cst = p_cst[name]
                re_f = cpool.tile([Ds + 1, 65], F32, tag=f"ref_{name}")
                nc.sync.dma_start(out=re_f[:], in_=cst["rhs_ext"][:])
                re_sb = cpool.tile([Ds + 1, 65], F16, tag=f"re_{name}")
                nc.vector.tensor_copy(out=re_sb[:], in_=re_f[:])
                rhs_ext[name] = re_sb

                av = cpool.tile([64, 3], F32, tag=f"av_{name}")
                nc.sync.dma_start(out=av[:], in_=cst["avec"][:])
                wv = cpool.tile([128, Dcat], F32, tag=f"wv_{name}")
                parts = [("WsT", Ds, 0, 0), ("WdT", 12, Ds, 1), ("WeT", de, Ds + 12, 2)]
                for wnm, wd, coff, acol in parts:
                    wt = cpool.tile([64, wd], F32, tag=f"wt_{name}_{wnm}")
                    nc.sync.dma_start(out=wt[:], in_=cst[wnm][:])
                    psw = spsp.tile([128, wd], F32, tag="setup")
                    nc.tensor.matmul(
                        out=psw[:],
                        lhsT=av[:, acol:acol + 1].to_broadcast([64, 128]),
                        rhs=wt[:], start=True, stop=True)
                    nc.vector.tensor_copy(out=wv[:, coff:coff + wd], in_=psw[:])
                wvec_rep[name] = wv

            spsp_cm.__exit__(None, None, None)

            # ---------------- main loop ----------------
            pspool_cm = tc.tile_pool(name="ps", bufs=1, space="PSUM")
            pspool = pspool_cm.__enter__()
            ps2pool_cm = tc.tile_pool(name="ps2", bufs=2, space="PSUM")
            ps2pool = ps2pool_cm.__enter__()
            for rnd in range(n_tiles // rt):
                x_rt = rpool.tile([128, rt * 12], F32, tag="x_rt")
                nc.sync.dma_start(
                    out=x_rt[:],
                    in_=p_x[:].rearrange("(t p) d -> p t d", p=128)[
                        :, rnd * rt:(rnd + 1) * rt, :])

                onehot, exv, rhs1 = {}, {}, {}
                for name, Ds, de in RELS:
                    Dcat = Ds + 12 + de
                    nb = NBs[name]
                    gb = rt * nb  # blocks this round
                    xc = rpool.tile([128, gb * Dcat], F32, tag=f"xc_{name}")
                    nc.sync.dma_start(
                        out=xc[:],
                        in_=p_xcat[name][:, rnd * gb * Dcat:(rnd + 1) * gb * Dcat])
                    dsl = rpool.tile([128, gb], F16, tag=f"ds_{name}")
                    nc.sync.dma_start(
                        out=dsl[:], in_=p_dslot[name][:, rnd * gb:(rnd + 1) * gb])

                    oh = rpool.tile([128, gb * 128], F16, tag=f"oh_{name}")
                    nc.vector.tensor_tensor(
                        out=oh[:].rearrange("p (g s) -> p g s", s=128),
                        in0=iota_sb[:].rearrange("p (o s) -> p o s", o=1).to_broadcast([128, gb, 128]),
                        in1=dsl[:].rearrange("p (g o) -> p g o", o=1).to_broadcast([128, gb, 128]),
                        op=AL.is_equal)
                    onehot[name] = oh

                    dtmp = rpool.tile([128, gb * Dcat], F32, tag=f"dt_{name}")
                    nc.vector.tensor_tensor(
                        out=dtmp[:],
                        in0=xc[:],
                        in1=wvec_rep[name][:].rearrange("p (o d) -> p o d", o=1).to_broadcast([128, gb, Dcat]),
                        op=AL.mult)
                    lg = rpool.tile([128, gb], F32, tag=f"lg_{name}")
                    nc.vector.tensor_reduce(
                        out=lg[:], in_=dtmp[:].rearrange("p (g d) -> p g d", d=Dcat),
                        axis=mybir.AxisListType.X, op=AL.add)
                    lg2 = rpool.tile([128, gb], F32, tag=f"lg2_{name}")
                    nc.vector.tensor_scalar(out=lg2[:], in0=lg[:], scalar1=0.2,
                                            scalar2=None, op0=AL.mult)
                    nc.vector.tensor_tensor(out=lg[:], in0=lg[:], in1=lg2[:], op=AL.max)
                    ex = rpool.tile([128, gb], F32, tag=f"ex_{name}")
                    nc.scalar.activation(out=ex[:], in_=lg[:], func=AF.Exp)
                    exv[name] = ex

                    r1 = rpool.tile([128, gb * (Ds + 1)], F16, tag=f"r1_{name}")
                    nc.vector.tensor_tensor(
                        out=r1[:].rearrange("p (g d) -> p g d", d=Ds + 1)[:, :, 0:Ds],
                        in0=xc[:].rearrange("p (g d) -> p g d", d=Dcat)[:, :, 0:Ds],
                        in1=ex[:].rearrange("p (g o) -> p g o", o=1).to_broadcast([128, gb, Ds]),
                        op=AL.mult)
                    nc.vector.tensor_copy(
                        out=r1[:].rearrange("p (g d) -> p g d", d=Ds + 1)[:, :, Ds:Ds + 1],
                        in_=ex[:].rearrange("p (g o) -> p g o", o=1))
                    rhs1[name] = r1

                for ti in range(rt):
                    t = rnd * rt + ti
                    # residual: x tile -> transpose -> [I|Wr;0|b] matmul
                    xtp = pspool.tile([12, 128], F32, tag="xtp")
                    nc.tensor.transpose(out=xtp[:], in_=x_rt[:, ti * 12:(ti + 1) * 12],
                                        identity=ident_sb[:])
                    xte = tpool.tile([13, 128], F16, tag="xte")
                    nc.vector.memset(xte[:], 1.0)
                    nc.vector.tensor_copy(out=xte[0:12, :], in_=xtp[:])
                    ocat = ps2pool.tile([128, DOUT], F32, tag="ocat")
                    nc.tensor.matmul(out=ocat[:], lhsT=xte[:], rhs=rr_sb[:],
                                     start=True, stop=True)

                    msg = ps2pool.tile([128, 195], F32, tag="msg")
                    for ri, (name, Ds, de) in enumerate(RELS):
                        nb = NBs[name]
                        at = pspool.tile([Ds + 1, 128], F32, tag=f"at_{name}")
                        for g in range(nb):
                            bi = ti * nb + g
                            nc.tensor.matmul(
                                out=at[:],
                                lhsT=rhs1[name][:, bi * (Ds + 1):(bi + 1) * (Ds + 1)],
                                rhs=onehot[name][:, bi * 128:(bi + 1) * 128],
                                start=(g == 0), stop=(g == nb - 1))
                        at_sb = tpool.tile([Ds + 1, 128], F16, tag=f"ats_{name}")
                        nc.vector.tensor_copy(out=at_sb[:], in_=at[:])
                        nc.tensor.matmul(
                            out=msg[:, 65 * ri:65 * ri + 65], lhsT=at_sb[:],
                            rhs=rhs_ext[name][:], start=True, stop=True,
                            skip_group_check=True)

                    # epilogue
                    xsb = tpool.tile([128, DOUT], F32, tag="xsb")
                    nc.vector.tensor_copy(out=xsb[:], in_=ocat[:])
                    d3 = tpool.tile([128, 3], F32, tag="d3")
                    nc.vector.tensor_scalar(
                        out=d3[:],
                        in0=msg[:].rearrange("p (r c) -> p r c", c=65)[:, :, 64:65],
                        scalar1=1e-16, scalar2=None, op0=AL.add)
                    rec = tpool.tile([128, 3], F32, tag="rec")
                    nc.vector.reciprocal(out=rec[:], in_=d3[:])
                    mscl = tpool.tile([128, 192], F32, tag="mscl")
                    nc.vector.tensor_tensor(
                        out=mscl[:].rearrange("p (r c) -> p r c", c=64),
                        in0=msg[:].rearrange("p (r c) -> p r c", c=65)[:, :, 0:64],
                        in1=rec[:].rearrange("p (r o) -> p r o", o=1).to_broadcast([128, 3, 64]),
                        op=AL.mult)
                    nc.vector.tensor_tensor(out=xsb[:, 12:DOUT], in0=xsb[:, 12:DOUT],
                                            in1=mscl[:], op=AL.add)

                    # layernorm + leaky_relu
                    sq = tpool.tile([128, DOUT], F32, tag="sq")
                    ssum = tpool.tile([128, 6], F32, tag="ssum")
                    nc.scalar.activation(out=sq[:], in_=xsb[:], func=AF.Square,
                                         accum_out=ssum[:, 1:2])
                    nc.vector.tensor_reduce(out=ssum[:, 0:1], in_=xsb[:],
                                            axis=mybir.AxisListType.X, op=AL.add)
                    # mu = s/204 ; m2 = sq/204 ; var = m2 - mu^2
                    nc.vector.tensor_scalar(out=ssum[:, 2:4], in0=ssum[:, 0:2],
                                            scalar1=1.0 / DOUT, scalar2=None, op0=AL.mult)
                    nc.vector.tensor_tensor(out=ssum[:, 4:5], in0=ssum[:, 2:3],
                                            in1=ssum[:, 2:3], op=AL.mult)
                    nc.vector.tensor_tensor(out=ssum[:, 4:5], in0=ssum[:, 3:4],
                                            in1=ssum[:, 4:5], op=AL.subtract)
                    nc.vector.tensor_scalar(out=ssum[:, 4:5], in0=ssum[:, 4:5],
                                            scalar1=1e-5, scalar2=None, op0=AL.add)
                    nc.scalar.activation(out=ssum[:, 5:6], in_=ssum[:, 4:5], func=AF.Sqrt)
                    rstd = tpool.tile([128, 2], F32, tag="rstd")
                    nc.vector.reciprocal(out=rstd[:, 0:1], in_=ssum[:, 5:6])
                    # negmurstd = -mu * rstd
                    nc.vector.tensor_tensor(out=rstd[:, 1:2], in0=ssum[:, 2:3],
                                            in1=rstd[:, 0:1], op=AL.mult)
                    nc.vector.tensor_scalar(out=rstd[:, 1:2], in0=rstd[:, 1:2],
                                            scalar1=-1.0, scalar2=None, op0=AL.mult)
                    yv = tpool.tile([128, DOUT], F32, tag="yv")
                    nc.scalar.activation(out=yv[:], in_=xsb[:], func=AF.Identity,
                                         scale=rstd[:, 0:1], bias=rstd[:, 1:2])
                    nc.gpsimd.tensor_tensor(out=yv[:], in0=yv[:], in1=g_sb[:], op=AL.mult)
                    nc.gpsimd.tensor_tensor(out=yv[:], in0=yv[:], in1=b_sb[:], op=AL.add)
                    yv2 = tpool.tile([128, DOUT], F32, tag="yv2")
                    nc.vector.tensor_scalar(out=yv2[:], in0=yv[:], scalar1=0.01,
                                            scalar2=None, op0=AL.mult)
                    nc.vector.tensor_tensor(out=yv[:], in0=yv[:], in1=yv2[:], op=AL.max)
                    nc.sync.dma_start(
                        out=p_out[:].rearrange("(t p) d -> p t d", p=128)[:, t, :],
                        in_=yv[:])
            ps2pool_cm.__exit__(None, None, None)
            pspool_cm.__exit__(None, None, None)

    nc.compile()
    return nc


# --------------------------------------------------------------------------
# entry point
# --------------------------------------------------------------------------

def kernel(**inputs) -> np.ndarray:
    in_maps, NBs = _prep_inputs(inputs)
    nc = build_program(NBs)
    res = bass_utils.run_bass_kernel_spmd(nc, in_maps, list(range(NCORES)))
    out = np.concatenate([res.results[k]["out"][:PT] for k in range(NCORES)], axis=0)
    return out.astype(np.float32)


if __name__ == "__main__":
    rng = np.random.default_rng(0)
    demo = {}
    demo["x_tasks"] = rng.standard_normal((N_TASKS, 12)).astype(np.float32)
    demo["x_data"] = rng.standard_normal((N_DATA, 5)).astype(np.float32)
    demo["x_devices"] = rng.standard_normal((N_DEV, 12)).astype(np.float32)
    sizes = {"td": (800000, N_DATA, 3), "tt": (1600000, N_TASKS, 1), "dv": (800000, N_DEV, 2)}
    for nm, (E, hi, de) in sizes.items():
        demo[f"src_{nm}"] = rng.integers(0, hi, E).astype(np.int32)
        demo[f"dst_{nm}"] = rng.integers(0, N_TASKS, E).astype(np.int32)
        demo[f"eattr_{nm}"] = rng.standard_normal((E, de)).astype(np.float32)
    for nm, Ds, de in RELS:
        demo[f"Ws_{nm}"] = (rng.standard_normal((Ds, C)) * 0.1).astype(np.float32)
        demo[f"Wd_{nm}"] = (rng.standard_normal((12, C)) * 0.1).astype(np.float32)
        demo[f"We_{nm}"] = (rng.standard_normal((de, C)) * 0.1).astype(np.float32)
        for v in ("as", "ad", "ae"):
            demo[f"{v}_{nm}"] = (rng.standard_normal(C) * 0.1).astype(np.float32)
        demo[f"Wr_{nm}"] = (rng.standard_normal((12, C)) * 0.1).astype(np.float32)
        demo[f"b_{nm}"] = np.zeros(C, np.float32)
    demo["ln_g"] = np.ones(DOUT, np.float32)
    demo["ln_b"] = np.zeros(DOUT, np.float32)
    out = kernel(**demo)
    print("out", out.shape, out.dtype, np.abs(out).mean())
